# revision 46
# baseline (speedup 1.0000x reference)
"""Trainium2 Bass kernel for nn_Encoder_71313636983306 (pillar scatter encoder).

Computes, for each (batch, frame) pair:
    emb = relu(BN(Linear(pcl))) * mask          # [N, 64] point embeddings
    grid = scatter_add(emb, cell_idx)           # [64, 640*640]
and returns the 4 grids stacked as [B*2, 64, 640, 640] (f32).

Sharding: 8 cores = 4 (batch, frame) pairs x 2 grid halves. Each core
processes the (unmasked) points of its pair that land in its half of the
640x640 grid and writes a dense [64, 204800] half-grid (f16; the host
upconverts -- tolerance is 2e-2, f16 keeps us ~1e-3).

Device algorithm (per core): the half-grid is covered by T tasks; task j
owns cells [Wh*j, Wh*j+Wh) ("A") and [102400 + Wh*j, +Wh) ("B"), Wh=WIN/2.
The host packs each task's points (<=128, checked) into 128 "slots".
Per task:
  1. pointnet: ONE f16 matmul, K=8: [x;y;z;1] stacked twice (A-rows for
     A-points, B-rows for B-points) against w8 [8, 128] -> PSUM
     [128slots, 128]: emb in columns 0:64 for A-points / 64:128 for
     B-points, bias folded in via the constant-1 coordinate row.
  2. relu: ScalarE PSUM->SBUF f16 (once per 4-task quad on [128, 512]).
  3. one-hot M[128slots, Wh] f16: GPSIMD local_scatter (int16 indices) or
     DVE is_equal(f16 iota, idx), alternating per M_PATTERN.
  4. grid matmul emb^T @ M (single f16 pass) -> PSUM [128, Wh]: rows 0:64
     = A-window channels, 64:128 = B-window channels (the A/B column
     gating in emb keeps the PE at full 128-lane output rate).
  5. once per quad: one [128, 4*Wh] copy PSUM -> f16 SBUF staging
     (DVE/ACT per COPY_PATTERN); every FLUSH_T tasks one ~1 MB DMA
     writes the staging buffer to HBM.

The loop batches GROUP quads of pointnet matmuls, then the grid matmuls
for the batch GROUP quads behind, so the PE stream is gapless (never
waits on the ACT relu) and pn<->grid weight-shape switches (~110ns PE
stall each) are halved. xt chunks prefetch PREF_Q quads ahead on the
Activation HWDGE queue, separate from output flushes on the sync queue.
"""
import numpy as np

# ---------------------------------------------------------------- constants
B = 2
D = 64
N_PX = N_PY = 640
P_CELLS = N_PX * N_PY          # 409600
HALF_CELLS = P_CELLS // 2      # 204800 cells per core
QH = HALF_CELLS // 2           # 102400: A/B half-of-half offset
NSLOT = 128                    # point slots per task
BN_EPS = 1e-5
N_CORES = 8

COPY_PATTERN = "vsvvs"         # stage-copy engine by quad (v=DVE, s=ACT)
M_PATTERN = "vgg"              # one-hot build engine by task (v=DVE, g=GPSIMD)
RELU_PATTERN = "s"             # relu engine by quad (s=ACT, v=DVE)
GROUP = 2                      # quads whose pn/grid matmuls are batched
N_WARM = 28                    # PE warm-up matmuls during the idle DMA head
PREF_Q = 6                     # quads of xt-chunk DMA prefetch distance
FIRST_CHUNK = 8                # tasks in the first (small, fast-start) chunk
CONFIGS = ((512, 96), (512, 128), (640, 128))  # (win, slots), overflow falls through

# per-WIN derived loop constants: tasks, xt-chunk tasks, flush tasks
_DERIVED = {640: dict(T=320, CHUNK_T=40, FLUSH_T=8),
            512: dict(T=400, CHUNK_T=40, FLUSH_T=8)}

_cached = {}


# ---------------------------------------------------------------- device code
def _build_kernel(win, nslot):
    from contextlib import ExitStack
    import concourse.tile as tile
    from concourse import bacc, mybir

    f32 = mybir.dt.float32
    bf16 = mybir.dt.bfloat16
    i16 = mybir.dt.int16
    f16 = mybir.dt.float16

    cfg = _DERIVED[win]
    T, CHUNK_T, FLUSH_T = cfg["T"], cfg["CHUNK_T"], cfg["FLUSH_T"]
    WH = win // 2
    # quad mode: one 2-bank PSUM tile holds 4 tasks' grid outputs and is
    # evacuated by a single [128, 4*WH] copy (needs 4*WH*4B <= 2 banks)
    quad = 4 * WH * 4 <= 4096

    nc = bacc.Bacc("TRN2", target_bir_lowering=False, debug=False,
                   num_devices=N_CORES)

    xt8 = nc.dram_tensor("xt8", [8, T * nslot], f16,
                         kind="ExternalInput").ap()
    scat = nc.dram_tensor("scat", [nslot, 2 * T], i16,
                          kind="ExternalInput").ap()
    idxc = nc.dram_tensor("idxc", [nslot, T], f32, kind="ExternalInput").ap()
    w8 = nc.dram_tensor("w8", [8, 2 * D], f16, kind="ExternalInput").ap()
    iota = nc.dram_tensor("iota", [nslot, WH], f16,
                          kind="ExternalInput").ap()
    # Output keeps the staging layout: row p = 64*h + d holds cells
    # [102400*h + WH*j, +WH) of task j; the host deinterleaves the halves.
    grid = nc.dram_tensor("grid", [2 * D, T * WH], f16,
                          kind="ExternalOutput").ap()

    # xt chunk schedule: a small first chunk so the PE starts early, then
    # CHUNK_T-task chunks; prefetched PREF_Q quads ahead of first use.
    sched = []
    t0 = 0
    while t0 < T:
        n = FIRST_CHUNK if t0 == 0 else min(CHUNK_T, T - t0)
        sched.append((t0, n))
        t0 += n
    task_chunk = {}
    for ci, (t0, n) in enumerate(sched):
        for t in range(t0, t0 + n):
            task_chunk[t] = ci

    with tile.TileContext(nc) as tc:
        with ExitStack() as ctx:
            consts = ctx.enter_context(tc.tile_pool(name="consts", bufs=1))
            xt_pool = ctx.enter_context(tc.tile_pool(name="xtc", bufs=4))
            emb_pool = ctx.enter_context(
                tc.tile_pool(name="emb", bufs=2 * GROUP + 2))
            m_pool = ctx.enter_context(
                tc.tile_pool(name="m", bufs=4 * (2 * GROUP + 2)))
            stage_pool = ctx.enter_context(tc.tile_pool(name="stage", bufs=4))
            pn_psum = ctx.enter_context(
                tc.tile_pool(name="pnps", bufs=4 if GROUP >= 4 else 2,
                             space="PSUM"))
            pair_w = 2 * WH if 2 * WH <= 512 else 1024
            if quad:
                gr_psum = ctx.enter_context(
                    tc.tile_pool(name="grps", bufs=2 if GROUP >= 4 else 3,
                                 space="PSUM"))
            else:
                gr_psum = ctx.enter_context(
                    tc.tile_pool(name="grps", bufs=4 if pair_w <= 512 else 2,
                                 space="PSUM"))

            chunks = {}

            def load_chunk(ci):
                if ci in chunks or ci >= len(sched):
                    return
                t0, n = sched[ci]
                xc = xt_pool.tile([8, n * nslot], f16, name="xc")
                nc.scalar.dma_start(
                    xc[:], xt8[:, t0 * nslot:(t0 + n) * nslot])
                chunks[ci] = xc

            # xt chunks load via the Activation HWDGE queue so they never
            # head-of-line block the output flushes on the sync queue; the
            # M-build consts go on the sync queue in parallel. chunk 0 +
            # w8 first: they gate the first pointnet matmul.
            w8_t = consts.tile([8, 2 * D], f16)
            nc.scalar.dma_start(w8_t[:], w8[:])
            load_chunk(0)
            iota_t = consts.tile([nslot, WH], f16)
            nc.sync.dma_start(iota_t[:], iota[:])
            scat_t = consts.tile([nslot, 2 * T], i16)
            nc.sync.dma_start(scat_t[:], scat[:])
            idxc_t = consts.tile([nslot, T], f32)
            nc.sync.dma_start(idxc_t[:], idxc[:])
            ones2 = consts.tile([nslot, 2], f16)
            nc.gpsimd.memset(ones2[:], 1.0)


            def emit_front(g):
                """M-builds + pointnet matmuls + relu for quad g."""
                j0 = 4 * g
                ci = task_chunk[j0]
                load_chunk(ci)
                xc = chunks[ci]
                pj = j0 + 4 * PREF_Q
                if pj in task_chunk and task_chunk[pj] != ci:
                    load_chunk(task_chunk[pj])

                m_ts = []
                for q in range(4):
                    j = j0 + q
                    m_t = m_pool.tile([nslot, WH], f16)
                    if M_PATTERN[j % len(M_PATTERN)] == "g":
                        nc.gpsimd.local_scatter(
                            m_t[:], ones2[:], scat_t[:, 2 * j:2 * j + 2],
                            channels=nslot, num_elems=WH, num_idxs=2)
                    else:
                        nc.vector.tensor_scalar(
                            m_t[:], iota_t[:], idxc_t[:, j:j + 1], None,
                            mybir.AluOpType.is_equal)
                    m_ts.append(m_t)

                pn = pn_psum.tile([nslot, 512], f32, space="PSUM")
                if g == 0:
                    # warm the PE clock during the otherwise-idle input-DMA
                    # head: junk K=8 matmuls (pointnet-shaped) into the
                    # first pn region; the real q=0 matmul (start=True)
                    # then overwrites it
                    for _ in range(N_WARM):
                        nc.tensor.matmul(pn[:, 0:2 * D],
                                         lhsT=w8_t[:, 0:nslot],
                                         rhs=w8_t[:], start=True, stop=True)
                for q in range(4):
                    jc = (j0 + q) - sched[ci][0]
                    nc.tensor.matmul(
                        pn[:, q * 2 * D:(q + 1) * 2 * D],
                        lhsT=xc[:, jc * nslot:(jc + 1) * nslot],
                        rhs=w8_t[:], start=True, stop=True)
                emb_h = emb_pool.tile([nslot, 512], f16, tag="embh")
                if RELU_PATTERN[g % len(RELU_PATTERN)] == "s":
                    nc.scalar.activation(
                        emb_h[:], pn[:], mybir.ActivationFunctionType.Relu)
                else:
                    nc.vector.tensor_scalar(
                        emb_h[:], pn[:], 0.0, None, mybir.AluOpType.max)
                return (j0, m_ts, emb_h)

            def emit_back(state):
                """Grid matmuls + stage copy (+flush) for a front state."""
                j0, m_ts, emb_h = state
                g = j0 // 4
                if j0 % FLUSH_T == 0:
                    back_state["stage"] = stage_pool.tile(
                        [2 * D, FLUSH_T * WH], f16, name="stage")
                stage = back_state["stage"]

                if quad:
                    gr = gr_psum.tile([2 * D, 4 * WH], f32, space="PSUM")
                    for q in range(4):
                        nc.tensor.matmul(
                            gr[:, q * WH:(q + 1) * WH],
                            lhsT=emb_h[:, q * 2 * D:(q + 1) * 2 * D],
                            rhs=m_ts[q][:], start=True, stop=True)
                    sdst = stage[:, (j0 % FLUSH_T) * WH:
                                 (j0 % FLUSH_T + 4) * WH]
                    if COPY_PATTERN[g % len(COPY_PATTERN)] == "v":
                        nc.vector.tensor_copy(sdst, gr[:])
                    else:
                        nc.scalar.copy(sdst, gr[:])
                else:
                    gr = None
                    for q in range(4):
                        j = j0 + q
                        if q % 2 == 0:
                            # pair tile: 2 banks, outs at col 0/512
                            gr = gr_psum.tile([2 * D, pair_w], f32,
                                              space="PSUM")
                        go = (q % 2) * (pair_w // 2)
                        nc.tensor.matmul(
                            gr[:, go:go + WH],
                            lhsT=emb_h[:, q * 2 * D:(q + 1) * 2 * D],
                            rhs=m_ts[q][:], start=True, stop=True)
                        if q % 2 == 1:
                            src = gr[:].rearrange("p (b c) -> p b c",
                                                  b=2)[:, :, 0:WH]
                            if pair_w == 2 * WH:
                                src = gr[:]
                            sdst = stage[:, (j % FLUSH_T - 1) * WH:
                                         (j % FLUSH_T + 1) * WH]
                            if pair_w != 2 * WH:
                                sdst = sdst.rearrange("p (b c) -> p b c",
                                                      b=2)
                            if COPY_PATTERN[(j // 2) %
                                            len(COPY_PATTERN)] == "v":
                                nc.vector.tensor_copy(sdst, src)
                            else:
                                nc.scalar.copy(sdst, src)

                if j0 % FLUSH_T == FLUSH_T - 4:
                    fl = j0 // FLUSH_T
                    nc.sync.dma_start(
                        grid[:, fl * FLUSH_T * WH:(fl + 1) * FLUSH_T * WH],
                        stage[:])

            back_state = {"stage": None}
            NQ = T // 4
            pending = []
            for bb in range(0, NQ + GROUP, GROUP):
                for g in range(bb, min(bb + GROUP, NQ)):
                    pending.append(emit_front(g))
                if bb >= GROUP:
                    for _ in range(min(GROUP, len(pending))):
                        emit_back(pending.pop(0))
            while pending:
                emit_back(pending.pop(0))

    nc.compile()
    return nc


def _get_nc(win, nslot):
    key = ("nc", win, nslot, M_PATTERN, COPY_PATTERN, RELU_PATTERN, GROUP,
           PREF_Q, FIRST_CHUNK, N_WARM)
    if key not in _cached:
        _cached[key] = _build_kernel(win, nslot)
    return _cached[key]


class _TaskOverflow(RuntimeError):
    pass


# ---------------------------------------------------------------- host prep
def _fold_bn(W, b, bn_gamma, bn_beta, bn_mean, bn_var):
    s = (bn_gamma / np.sqrt(bn_var + np.float32(BN_EPS))).astype(np.float32)
    Wp = (W * s[:, None]).T.astype(np.float32)            # [3, 64]
    bp = ((b - bn_mean) * s + bn_beta).astype(np.float32)  # [64]
    w8 = np.zeros((8, NSLOT), np.float32)
    w8[0:3, 0:D] = Wp
    w8[3, 0:D] = bp
    w8[4:7, D:2 * D] = Wp
    w8[7, D:2 * D] = bp
    return w8.astype(np.float16)


def _prep_core(pcl, mask, idx, half, win, nslot):
    """Pack one core's points into the task layout. Raises on task overflow."""
    T = _DERIVED[win]["T"]
    WH = win // 2
    lo_cell = half * HALF_CELLS
    idx = idx.astype(np.int64)
    keep = mask & (idx >= lo_cell) & (idx < lo_cell + HALF_CELLS)
    il = idx[keep] - lo_cell
    pts = pcl[keep].astype(np.float32)

    # task j owns cells [WH*j, +WH) (A) and [102400 + WH*j, +WH) (B)
    tid = (il % QH) // WH
    order = np.argsort(tid, kind="stable")
    il = il[order]
    pts = pts[order]
    tid = tid[order]
    cloc = (il % QH) - tid * WH              # local cell within WH-window
    rowbase = (il >= QH) * 4                 # 0 for half A, 4 for half B
    counts = np.bincount(tid, minlength=T)
    if counts.max() > nslot:
        raise _TaskOverflow(
            f"{counts.max()} points > {nslot} slots in one {win}-cell window")
    starts = np.zeros(T + 1, np.int64)
    np.cumsum(counts, out=starts[1:])
    slot = np.arange(len(il)) - starts[tid]
    col = tid * nslot + slot

    xt = np.zeros((8, T * nslot), np.float32)
    xt[rowbase, col] = pts[:, 0]
    xt[rowbase + 1, col] = pts[:, 1]
    xt[rowbase + 2, col] = pts[:, 2]
    xt[rowbase + 3, col] = 1.0
    scat = np.full((nslot, 2 * T), -1, np.int16)
    scat[slot, 2 * tid] = cloc.astype(np.int16)
    idxcol = np.full((nslot, T), -1.0, np.float32)
    idxcol[slot, tid] = cloc.astype(np.float32)
    return xt.astype(np.float16), scat, idxcol, counts


def make_in_maps(win, nslot, previous_pcl, previous_mask, previous_grid,
                 current_pcl, current_mask, current_grid,
                 W, b, bn_gamma, bn_beta, bn_mean, bn_var):
    w8 = _fold_bn(np.asarray(W), np.asarray(b), np.asarray(bn_gamma),
                  np.asarray(bn_beta), np.asarray(bn_mean),
                  np.asarray(bn_var))
    iota = np.tile(np.arange(win // 2, dtype=np.float16), (nslot, 1))
    frames = [
        (np.asarray(previous_pcl), np.asarray(previous_mask),
         np.asarray(previous_grid)),
        (np.asarray(current_pcl), np.asarray(current_mask),
         np.asarray(current_grid)),
    ]
    in_maps = []
    kmax = None
    for core in range(N_CORES):
        q = core // 2          # pair: q = 2*b + frame
        bb, fr = q // 2, q % 2
        pcl, mask, gidx = frames[fr]
        xt8, scat, idxcol, cnt = _prep_core(
            pcl[bb], np.asarray(mask[bb], bool), gidx[bb],
            core % 2, win, nslot)
        in_maps.append({"xt8": xt8, "scat": scat, "idxc": idxcol,
                        "w8": w8, "iota": iota})
    return in_maps


def assemble_output(results):
    out = np.empty((B * 2, D, P_CELLS), np.float32)
    for q in range(B * 2):
        for h in range(2):
            dev = results[2 * q + h]["grid"]       # [128, 102400] f16
            lo = h * HALF_CELLS
            out[q, :, lo:lo + QH] = dev[:D]
            out[q, :, lo + QH:lo + HALF_CELLS] = dev[D:]
    return out.reshape(B * 2, D, N_PX, N_PY)


# ---------------------------------------------------------------- entry point
def kernel(previous_pcl, previous_mask, previous_grid,
           current_pcl, current_mask, current_grid,
           W, b, bn_gamma, bn_beta, bn_mean, bn_var,
           _trace=False, _trace_cores=None):
    from concourse.bass_utils import run_bass_kernel_spmd

    kw = dict(previous_pcl=previous_pcl, previous_mask=previous_mask,
              previous_grid=previous_grid, current_pcl=current_pcl,
              current_mask=current_mask, current_grid=current_grid,
              W=W, b=b, bn_gamma=bn_gamma, bn_beta=bn_beta,
              bn_mean=bn_mean, bn_var=bn_var)
    in_maps = None
    for i, (win, nslot) in enumerate(CONFIGS):
        try:
            in_maps = make_in_maps(win, nslot, **kw)
            break
        except _TaskOverflow:
            if i == len(CONFIGS) - 1:
                raise
    nc = _get_nc(win, nslot)
    res = run_bass_kernel_spmd(nc, in_maps, core_ids=list(range(N_CORES)),
                               trace=_trace, trace_cores=_trace_cores)
    out = assemble_output(res.results)
    if _trace:
        _cached["last_result"] = res
    return out
